# revision 10
# baseline (speedup 1.0000x reference)
"""Multi-head attention forward on 8 Trainium2 NeuronCores.

Sharding: core = (batch b in 0..2, head-group hg in 0..4); each core owns
4 of the 16 heads for one batch element. Q/K/V projections are computed
per-core for its 256 head-dims; attention runs per head with scores kept
transposed (S^T[k, q]); the output projection is row-sharded over W_o,
producing a per-core partial Y that the host sums over the 4 head-groups
of each batch.

v2: single fully-pipelined schedule. The softmax exp stream on ScalarE
(16.8M elements/core at 1 elem/cycle/lane ~= 137us) is the critical
resource, so the kernel is ordered to start it as early as possible
(K proj -> Q proj j0 -> first scores at ~20us) and to fill the PE's
spare cycles underneath it with the V projection, remaining Q slices,
and the output projection (emitted per q-slice as soon as both head
pairs are normalized). Even/odd heads of a pair are issued as adjacent
row-tiled score matmuls (contraction=64 -> tile_position (0,0)/(64,0))
so the PE array runs both concurrently. All PSUM->SBUF copies run on
the DVE so ScalarE does nothing but exp. Output is written fp16.

All matmul operands are fp16 (PSUM accumulation stays fp32). V is stored
in head-pair blocks [V_even | ones | junk | V_odd] (192 cols); the PV
stationary is the 128-wide window starting at offset 0 (even head: ctx
rows 0:64, denom row 64) or offset 64 (odd head: denom row 0, ctx rows
64:128), so each head's softmax denominator comes free.
"""

import sys

for _p in ("/opt/trn_rl_repo", "/opt/pypackages"):
    if _p not in sys.path:
        sys.path.append(_p)

from contextlib import ExitStack

import numpy as np

import concourse.bass as bass
import concourse.tile as tile
from concourse import bacc, mybir
from concourse import bass_utils

P = 128
B = 2
S = 2048          # sequence length
D = 1024          # model dim
H = 16            # total heads
DK = 64           # head dim
HL = 4            # heads per core
CL = HL * DK      # local head dims per core (256)
NJ = 4            # 512-wide s-slices
NS = 512
NI = D // P       # 8 contraction tiles over model dim
NK = S // P       # 16 key tiles
VPB = 192         # V pair block: V_even(64) | ones(1) | junk(63) | V_odd(64)
VPAD = 2 * VPB    # 384 cols for 2 head pairs

F32 = mybir.dt.float32
F16 = mybir.dt.float16
EXP = mybir.ActivationFunctionType.Exp

# k-tile groups per (head-pair, q-slice): scores for a group land in one
# PSUM slot and get one exp instruction per head ([128, len*512]).
K_GROUPS = [(0,), (1, 2, 3), (4, 5, 6), (7, 8, 9), (10, 11, 12), (13, 14, 15)]


def build_nc():
    nc = bacc.Bacc("TRN2", target_bir_lowering=False, debug=False)

    xqT = nc.dram_tensor("xqT", [D, S], F16, kind="ExternalInput")
    xkT = nc.dram_tensor("xkT", [D, S], F16, kind="ExternalInput")
    xvT = nc.dram_tensor("xvT", [D, S], F16, kind="ExternalInput")
    wqT = nc.dram_tensor("wqT", [D, CL], F16, kind="ExternalInput")
    wkT = nc.dram_tensor("wkT", [D, CL], F16, kind="ExternalInput")
    wvT = nc.dram_tensor("wvT", [D, CL], F16, kind="ExternalInput")
    woT = nc.dram_tensor("woT", [CL, D], F16, kind="ExternalInput")
    y = nc.dram_tensor("y", [S, D], F16, kind="ExternalOutput")

    # Alternate big input DMAs across the two queue engines so both HBM
    # streams run concurrently.
    _dq = [None]

    def dq():
        _dq[0] = (_dq[0] or 0) + 1
        return nc.sync if _dq[0] % 2 == 0 else nc.gpsimd

    with tile.TileContext(nc) as tc, ExitStack() as ctx:
        wpool = ctx.enter_context(tc.tile_pool(name="w", bufs=1))
        big = ctx.enter_context(tc.tile_pool(name="big", bufs=1))
        xpool = ctx.enter_context(tc.tile_pool(name="xs", bufs=24))
        epool = ctx.enter_context(tc.tile_pool(name="ex", bufs=16))
        spool = ctx.enter_context(tc.tile_pool(name="sm", bufs=4))
        ypool = ctx.enter_context(tc.tile_pool(name="yo", bufs=3))
        psS = ctx.enter_context(tc.tile_pool(name="psS", bufs=2, space="PSUM"))
        psC = ctx.enter_context(tc.tile_pool(name="psC", bufs=2, space="PSUM"))

        # Resident weights
        wq_sb = wpool.tile([P, NI, CL], F16)
        wk_sb = wpool.tile([P, NI, CL], F16)
        wv_sb = wpool.tile([P, NI, CL], F16)
        wo_sb = wpool.tile([P, CL // P, D], F16)

        # Resident activations: K^T with head dims on partitions
        # ([128, ot, s]); per-j Q^T and normalized ctx^T tiles; per-k V
        # tiles in head-pair blocks (see module docstring).
        kT_sb = big.tile([P, 2, S], F16)
        qT_j = [big.tile([P, 2, NS], F16, tag=f"qT{j}", name=f"qT{j}")
                for j in range(NJ)]
        cT_j = [big.tile([P, 2, NS], F16, tag=f"cT{j}", name=f"cT{j}")
                for j in range(NJ)]
        vt = [big.tile([P, VPAD], F16, tag=f"v{k}", name=f"v{k}")
              for k in range(NK)]

        # ones + junk columns of each V pair block (cols 64:128, 256:320)
        for k in range(NK):
            nc.vector.memset(
                vt[k][:].rearrange("p (b c) -> p b c", c=VPB)[:, :, DK:2 * DK],
                1.0,
            )

        # ---- weight DMAs (gpsimd) + K/Q input DMAs (both queues) ------
        nc.gpsimd.dma_start(wk_sb[:], wkT.ap().rearrange("(n p) o -> p n o", p=P))
        nc.sync.dma_start(wq_sb[:], wqT.ap().rearrange("(n p) o -> p n o", p=P))

        def dma_x(x_dram, jh):
            xt = []
            for i in range(NI):
                t = xpool.tile([P, 2, NS], F16, tag="x")
                dq().dma_start(
                    t[:],
                    x_dram.ap()[i * P:(i + 1) * P, jh * 2 * NS:(jh * 2 + 2) * NS]
                    .rearrange("p (a s) -> p a s", s=NS),
                )
                xt.append(t)
            return xt

        xk = [dma_x(xkT, 0), dma_x(xkT, 1)]

        # ---- K projection ---------------------------------------------
        def proj_T(xt_jh, w_sb, jh, out_tile, out_is_slice):
            # out = (X @ W.T)^T for this core's 256 dims, one j-slice
            for jj in range(2):
                j = jh * 2 + jj
                sp = psS.tile([P, 3, NS], F32, tag="ps", name="pj")
                for i in range(NI):
                    for ot in range(2):
                        nc.tensor.matmul(
                            sp[:, ot],
                            w_sb[:, i, ot * P:(ot + 1) * P],
                            xt_jh[i][:, jj],
                            start=(i == 0),
                            stop=(i == NI - 1),
                        )
                if out_is_slice:
                    dst = out_tile[:, :, j * NS:(j + 1) * NS]
                else:
                    dst = out_tile[j][:, :, :]
                nc.vector.tensor_copy(dst, sp[:, 0:2, :])

        proj_T(xk[0], wk_sb, 0, kT_sb, True)
        proj_T(xk[1], wk_sb, 1, kT_sb, True)

        # ---- Q j0/j1 inputs + projection ------------------------------
        xq0 = dma_x(xqT, 0)
        nc.sync.dma_start(wv_sb[:], wvT.ap().rearrange("(n p) o -> p n o", p=P))
        proj_T(xq0, wq_sb, 0, qT_j, False)

        # ---- V inputs / wo --------------------------------------------
        xv = [dma_x(xvT, 0)]
        nc.gpsimd.dma_start(wo_sb[:], woT.ap().rearrange("(n p) o -> p n o", p=P))
        xv.append(dma_x(xvT, 1))

        # ---- attention machinery --------------------------------------
        pending = []

        def flush_one():
            fns = pending.pop(0)
            for fn in fns:
                fn()

        def emit_v_proj(jh):
            # V in natural [s, d] layout, packed into head-pair blocks
            for sbp in range(4):
                sp = psS.tile([P, 3, NS], F32, tag="ps", name="pv")
                for i in range(NI):
                    xf = xv[jh][i][:].rearrange("p a s -> p (a s)")
                    for u in range(2):
                        sb = sbp * 2 + u
                        nc.tensor.matmul(
                            sp[:, u, 0:CL],
                            xf[:, sb * P:(sb + 1) * P],
                            wv_sb[:, i, :],
                            start=(i == 0),
                            stop=(i == NI - 1),
                        )
                for u in range(2):
                    st = jh * 8 + sbp * 2 + u
                    vv = vt[st][:].rearrange("p (pr c) -> p pr c", c=VPB)
                    pv = sp[:, u, 0:CL].rearrange("p (pr hc) -> p pr hc", hc=2 * DK)
                    nc.vector.tensor_copy(vv[:, :, 0:DK], pv[:, :, 0:DK])
                    nc.vector.tensor_copy(vv[:, :, 2 * DK:3 * DK], pv[:, :, DK:2 * DK])

        def emit_block(ot, j, flush=True):
            # scores + exp for both heads of pair `ot`, q-slice j; PV and
            # normalization are deferred through `pending` so they fill
            # the PE while ScalarE streams exps.
            ctx_ps = {}

            def alloc_ctx():
                for pr in range(2):
                    ctx_ps[pr] = psC.tile([P, NS], F32, tag="ctx", name="ctx")

            for gi, ks in enumerate(K_GROUPS):
                ng = len(ks)
                sps = [psS.tile([P, 3, NS], F32, tag="ps", name="sc")
                       for _ in range(2)]
                for idx, k in enumerate(ks):
                    for pr in range(2):
                        pr0 = pr * 64
                        nc.tensor.matmul(
                            sps[pr][:, idx],
                            kT_sb[pr0:pr0 + 64, ot, k * P:(k + 1) * P],
                            qT_j[j][pr0:pr0 + 64, ot, :],
                            start=True,
                            stop=True,
                        )
                exs = []
                for pr in range(2):
                    ex = epool.tile([P, 3, NS], F16, tag="ex", name="ex")
                    nc.scalar.activation(
                        ex[:, 0:ng], sps[pr][:, 0:ng], EXP, scale=0.125,
                    )
                    exs.append(ex)

                def pv_fn(ks=ks, exs=exs, ot=ot, first=(gi == 0)):
                    if first:
                        alloc_ctx()
                    for pr in range(2):
                        vcol = ot * VPB + pr * DK
                        for idx, k in enumerate(ks):
                            nc.tensor.matmul(
                                ctx_ps[pr][:],
                                vt[k][:, vcol:vcol + P],
                                exs[pr][:, idx],
                                start=(k == 0),
                                stop=(k == NK - 1),
                            )

                fns = [pv_fn]
                if gi == len(K_GROUPS) - 1:
                    def norm_fn(ot=ot, j=j):
                        for pr in range(2):
                            pr0 = pr * 64
                            drow = 64 * (1 - pr)
                            cps = ctx_ps[pr]
                            den = spool.tile([1, NS], F32, tag="den", name="den")
                            nc.vector.tensor_copy(den[:], cps[drow:drow + 1, :])
                            rec = spool.tile([1, NS], F32, tag="rec", name="rec")
                            nc.vector.reciprocal_approx_fast(rec[:], den[:])
                            bc = spool.tile([P, NS], F32, tag="bc", name="bc")
                            nc.gpsimd.partition_broadcast(bc[:], rec[:])
                            nc.vector.tensor_mul(
                                cT_j[j][pr0:pr0 + 64, ot, :],
                                cps[pr0:pr0 + 64, :],
                                bc[pr0:pr0 + 64, :],
                            )
                    fns.append(norm_fn)
                pending.append(fns)
                # PVs of earlier groups fill the PE under the exp stream.
                # Block (0,0) must not flush: its PVs read V tiles whose
                # producing copies are constructed only after this block.
                while flush and len(pending) > 2:
                    flush_one()

        def emit_y(j):
            # output projection for q-slice j (needs both ot's of cT_j[j])
            for qb in range(4):
                ysb = ypool.tile([P, D], F16, tag="y")
                yp = psS.tile([P, 3, NS], F32, tag="ps", name="yp")
                for ct in range(2):
                    for oh in range(2):
                        nc.tensor.matmul(
                            yp[:, oh],
                            cT_j[j][:, ct, qb * P:(qb + 1) * P],
                            wo_sb[:, ct, oh * NS:(oh + 1) * NS],
                            start=(ct == 0),
                            stop=(ct == 1),
                        )
                nc.vector.tensor_copy(
                    ysb[:].rearrange("p (a s) -> p a s", s=NS), yp[:, 0:2, :]
                )
                dq().dma_start(y.ap()[(j * 4 + qb) * P:(j * 4 + qb + 1) * P, :],
                               ysb[:])

        # ---- pipelined schedule ---------------------------------------
        emit_block(0, 0, flush=False)  # scores+exp start once K, Q j0 ready
        emit_v_proj(0)            # PE fill under the first exps
        emit_v_proj(1)
        emit_block(1, 0)          # + flushes PVs of (0,0)
        xq1 = dma_x(xqT, 1)
        proj_T(xq1, wq_sb, 1, qT_j, False)
        emit_block(0, 1)          # + flushes PVs of (1,0)
        emit_block(1, 1)
        emit_y(0)
        emit_block(0, 2)
        emit_block(1, 2)
        emit_y(1)
        emit_block(0, 3)
        emit_block(1, 3)
        emit_y(2)
        while pending:
            flush_one()
        emit_y(3)

    nc.compile()
    return nc


_NC = None


def _get_nc():
    global _NC
    if _NC is None:
        _NC = build_nc()
    return _NC


def _shard_inputs(Query, Key, Value, W_q, W_k, W_v, W_o):
    in_maps = []
    xT = {}
    for b in range(B):
        xT[b] = (
            np.ascontiguousarray(Query[b].T).astype(np.float16),
            np.ascontiguousarray(Key[b].T).astype(np.float16),
            np.ascontiguousarray(Value[b].T).astype(np.float16),
        )
    for b in range(B):
        for hg in range(4):
            r0 = hg * CL
            in_maps.append({
                "xqT": xT[b][0],
                "xkT": xT[b][1],
                "xvT": xT[b][2],
                "wqT": np.ascontiguousarray(W_q[r0:r0 + CL, :].T).astype(np.float16),
                "wkT": np.ascontiguousarray(W_k[r0:r0 + CL, :].T).astype(np.float16),
                "wvT": np.ascontiguousarray(W_v[r0:r0 + CL, :].T).astype(np.float16),
                "woT": np.ascontiguousarray(W_o[:, r0:r0 + CL].T).astype(np.float16),
            })
    return in_maps


def _reference_np(Query, Key, Value, mask, W_q, W_k, W_v, W_o):
    # Fallback for a non-trivial mask (never hit for the spec'd inputs).
    out = np.empty((B, S, D), dtype=np.float32)
    m = np.broadcast_to(mask, (1, 1, S, S))[0, 0]
    for b in range(B):
        Q = (Query[b] @ W_q.T).reshape(S, H, DK).transpose(1, 0, 2)
        K = (Key[b] @ W_k.T).reshape(S, H, DK).transpose(1, 0, 2)
        V = (Value[b] @ W_v.T).reshape(S, H, DK).transpose(1, 0, 2)
        ctx = np.empty((H, S, DK), dtype=np.float32)
        for h in range(H):
            s = (Q[h] @ K[h].T) / np.sqrt(DK)
            s = np.where(m == 0, -1e9, s)
            s -= s.max(axis=-1, keepdims=True)
            e = np.exp(s)
            ctx[h] = (e / e.sum(axis=-1, keepdims=True)) @ V[h]
        out[b] = ctx.transpose(1, 0, 2).reshape(S, D) @ W_o.T
    return out


def kernel(Query, Key, Value, mask, W_q, W_k, W_v, W_o, **_ignored):
    Query = np.asarray(Query, dtype=np.float32)
    Key = np.asarray(Key, dtype=np.float32)
    Value = np.asarray(Value, dtype=np.float32)
    W_q = np.asarray(W_q, dtype=np.float32)
    W_k = np.asarray(W_k, dtype=np.float32)
    W_v = np.asarray(W_v, dtype=np.float32)
    W_o = np.asarray(W_o, dtype=np.float32)

    if not np.all(np.asarray(mask) != 0):
        return _reference_np(Query, Key, Value, np.asarray(mask),
                             W_q, W_k, W_v, W_o)

    nc = _get_nc()
    in_maps = _shard_inputs(Query, Key, Value, W_q, W_k, W_v, W_o)
    res = bass_utils.run_bass_kernel_spmd(nc, in_maps, core_ids=list(range(8)))
    out = np.zeros((B, S, D), dtype=np.float32)
    for b in range(B):
        for hg in range(4):
            out[b] += res.results[b * 4 + hg]["y"].astype(np.float32)
    return out


# revision 11
# speedup vs baseline: 1.0020x; 1.0020x over previous
"""Multi-head attention forward on 8 Trainium2 NeuronCores.

Sharding: core = (batch b in 0..2, head-group hg in 0..4); each core owns
4 of the 16 heads for one batch element. Q/K/V projections are computed
per-core for its 256 head-dims; attention runs per head with scores kept
transposed (S^T[k, q]); the output projection is row-sharded over W_o,
producing a per-core partial Y that the host sums over the 4 head-groups
of each batch.

v2.1: single fully-pipelined schedule. The softmax exp stream on ScalarE
(16.8M elements/core at 1 elem/cycle/lane ~= 137us) is the critical
resource, so the program is constructed in fine producer->consumer order
to start that stream as early as possible (~21us) and keep it dense:
K proj j0/j1 -> Q proj j0 -> score groups G0-G2 of the first head pair,
then K j2/j3 -> G3-G5, with the V projection and the remaining Q slices
woven underneath the exp stream, and the output projection emitted per
q-slice as soon as both head pairs are normalized. PSUM runs as one
2-deep ring of [128,3,512] slots (6 banks) shared by projections,
scores, and the output projection - the construction order guarantees
the ring's 2-back dependency is always a fast consumer (a DVE copy or
an exp) - plus 2 ctx banks. Even/odd heads of a pair are issued as
adjacent row-tiled score matmuls (contraction=64 -> tile_position
(0,0)/(64,0)) so the PE array can run both concurrently. All PSUM->SBUF
copies run on the DVE; weights are pre-interleaved on the host so their
DMAs are contiguous; output is written fp16 (summed in f32 on host).

All matmul operands are fp16 (PSUM accumulation stays fp32). V is stored
per k-tile in head-pair blocks [V_even | ones | junk | V_odd] (192 cols);
the PV stationary is the 128-wide window at offset 0 (even head: ctx
rows 0:64, denom row 64) or offset 64 (odd head: denom row 0, ctx rows
64:128), so each head's softmax denominator comes free.
"""

import sys

for _p in ("/opt/trn_rl_repo", "/opt/pypackages"):
    if _p not in sys.path:
        sys.path.append(_p)

from contextlib import ExitStack

import numpy as np

import concourse.bass as bass
import concourse.tile as tile
from concourse import bacc, mybir
from concourse import bass_utils

P = 128
B = 2
S = 2048          # sequence length
D = 1024          # model dim
H = 16            # total heads
DK = 64           # head dim
HL = 4            # heads per core
CL = HL * DK      # local head dims per core (256)
NJ = 4            # 512-wide s-slices
NS = 512
NI = D // P       # 8 contraction tiles over model dim
NK = S // P       # 16 key tiles
VPB = 192         # V pair block: V_even(64) | ones(1) | junk(63) | V_odd(64)
VPAD = 2 * VPB    # 384 cols for 2 head pairs

F32 = mybir.dt.float32
F16 = mybir.dt.float16
EXP = mybir.ActivationFunctionType.Exp

# k-tile groups per (head-pair, q-slice): scores for a group land in one
# PSUM slot and get one exp instruction per head ([128, len*512]).
K_GROUPS = [(0,), (1, 2, 3), (4, 5, 6), (7, 8, 9), (10, 11, 12), (13, 14, 15)]
# last vp chunk (writes vt[2c], vt[2c+1]) needed by each PV group
VP_NEED = [0, 1, 3, 4, 6, 7]


def build_nc():
    nc = bacc.Bacc("TRN2", target_bir_lowering=False, debug=False)

    xqT = nc.dram_tensor("xqT", [D, S], F16, kind="ExternalInput")
    xkT = nc.dram_tensor("xkT", [D, S], F16, kind="ExternalInput")
    xvT = nc.dram_tensor("xvT", [D, S], F16, kind="ExternalInput")
    # weights pre-interleaved on host to [128, n, out] layout
    wqT = nc.dram_tensor("wqT", [P, NI * CL], F16, kind="ExternalInput")
    wkT = nc.dram_tensor("wkT", [P, NI * CL], F16, kind="ExternalInput")
    wvT = nc.dram_tensor("wvT", [P, NI * CL], F16, kind="ExternalInput")
    woT = nc.dram_tensor("woT", [P, (CL // P) * D], F16, kind="ExternalInput")
    y = nc.dram_tensor("y", [S, D], F16, kind="ExternalOutput")

    _dq = [0]

    def dq():
        _dq[0] += 1
        return nc.sync if _dq[0] % 2 == 0 else nc.gpsimd

    with tile.TileContext(nc) as tc, ExitStack() as ctx:
        wpool = ctx.enter_context(tc.tile_pool(name="w", bufs=1))
        big = ctx.enter_context(tc.tile_pool(name="big", bufs=1))
        xpool = ctx.enter_context(tc.tile_pool(name="xs", bufs=24))
        epool = ctx.enter_context(tc.tile_pool(name="ex", bufs=16))
        spool = ctx.enter_context(tc.tile_pool(name="sm", bufs=4))
        ypool = ctx.enter_context(tc.tile_pool(name="yo", bufs=3))
        psS = ctx.enter_context(tc.tile_pool(name="psS", bufs=2, space="PSUM"))
        psC = ctx.enter_context(tc.tile_pool(name="psC", bufs=2, space="PSUM"))

        # Resident weights
        wq_sb = wpool.tile([P, NI, CL], F16)
        wk_sb = wpool.tile([P, NI, CL], F16)
        wv_sb = wpool.tile([P, NI, CL], F16)
        wo_sb = wpool.tile([P, CL // P, D], F16)

        kT_sb = big.tile([P, 2, S], F16)
        qT_j = [big.tile([P, 2, NS], F16, tag=f"qT{j}", name=f"qT{j}")
                for j in range(NJ)]
        cT_j = [big.tile([P, 2, NS], F16, tag=f"cT{j}", name=f"cT{j}")
                for j in range(NJ)]
        vt = [big.tile([P, VPAD], F16, tag=f"v{k}", name=f"v{k}")
              for k in range(NK)]

        # ones + junk columns of each V pair block (cols 64:128, 256:320)
        for k in range(NK):
            nc.vector.memset(
                vt[k][:].rearrange("p (b c) -> p b c", c=VPB)[:, :, DK:2 * DK],
                1.0,
            )

        # ---- DMA issue helpers ----------------------------------------
        def dma_w(dst, src):
            dq().dma_start(
                dst[:].rearrange("p n o -> p (n o)"), src.ap()
            )

        def dma_x(x_dram, jh):
            xt = []
            for i in range(NI):
                t = xpool.tile([P, 2, NS], F16, tag="x", name="xt")
                dq().dma_start(
                    t[:],
                    x_dram.ap()[i * P:(i + 1) * P, jh * 2 * NS:(jh * 2 + 2) * NS]
                    .rearrange("p (a s) -> p a s", s=NS),
                )
                xt.append(t)
            return xt

        # ---- compute emit helpers -------------------------------------
        def proj_j(xt_jh, w_sb, jj, dst):
            # dst <- (X @ W.T)^T for one 512-wide s-slice ([128, 2, 512])
            sp = psS.tile([P, 3, NS], F32, tag="ps", name="pj")
            for i in range(NI):
                for ot in range(2):
                    nc.tensor.matmul(
                        sp[:, ot],
                        w_sb[:, i, ot * P:(ot + 1) * P],
                        xt_jh[i][:, jj],
                        start=(i == 0),
                        stop=(i == NI - 1),
                    )
            nc.vector.tensor_copy(dst, sp[:, 0:2, :])

        xv = [None, None]

        def vp(c):
            # V-proj chunk c: 128x256 projection for s-tiles 2c, 2c+1,
            # packed into vt[2c], vt[2c+1] head-pair blocks.
            jh, sbp = divmod(c, 4)
            sp = psS.tile([P, 3, NS], F32, tag="ps", name="pv")
            for i in range(NI):
                xf = xv[jh][i][:].rearrange("p a s -> p (a s)")
                for u in range(2):
                    sb = sbp * 2 + u
                    nc.tensor.matmul(
                        sp[:, u, 0:CL],
                        xf[:, sb * P:(sb + 1) * P],
                        wv_sb[:, i, :],
                        start=(i == 0),
                        stop=(i == NI - 1),
                    )
            for u in range(2):
                st = 2 * c + u
                vv = vt[st][:].rearrange("p (pr c) -> p pr c", c=VPB)
                pv_ = sp[:, u, 0:CL].rearrange("p (pr hc) -> p pr hc", hc=2 * DK)
                nc.vector.tensor_copy(vv[:, :, 0:DK], pv_[:, :, 0:DK])
                nc.vector.tensor_copy(vv[:, :, 2 * DK:3 * DK], pv_[:, :, DK:2 * DK])

        pending = []

        def flush_one():
            fns = pending.pop(0)
            for fn in fns:
                fn()

        def sc_group(ot, j, gi, ctx_ps):
            # paired even/odd score matmuls + one exp per head; PV (and
            # final normalization) deferred via `pending`.
            ks = K_GROUPS[gi]
            ng = len(ks)
            sps = [psS.tile([P, 3, NS], F32, tag="ps", name="sc")
                   for _ in range(2)]
            for idx, k in enumerate(ks):
                for pr in range(2):
                    pr0 = pr * 64
                    nc.tensor.matmul(
                        sps[pr][:, idx],
                        kT_sb[pr0:pr0 + 64, ot, k * P:(k + 1) * P],
                        qT_j[j][pr0:pr0 + 64, ot, :],
                        start=True,
                        stop=True,
                    )
            exs = []
            for pr in range(2):
                ex = epool.tile([P, 3, NS], F16, tag="ex", name="ex")
                nc.scalar.activation(ex[:, 0:ng], sps[pr][:, 0:ng], EXP,
                                     scale=0.125)
                exs.append(ex)

            def pv_fn(ks=ks, exs=exs, ot=ot, first=(gi == 0)):
                if first:
                    for pr in range(2):
                        ctx_ps[pr] = psC.tile([P, NS], F32, tag="ctx",
                                              name="ctx")
                for pr in range(2):
                    vcol = ot * VPB + pr * DK
                    for idx, k in enumerate(ks):
                        nc.tensor.matmul(
                            ctx_ps[pr][:],
                            vt[k][:, vcol:vcol + P],
                            exs[pr][:, idx],
                            start=(k == 0),
                            stop=(k == NK - 1),
                        )

            fns = [pv_fn]
            if gi == len(K_GROUPS) - 1:
                def norm_fn(ot=ot, j=j):
                    for pr in range(2):
                        pr0 = pr * 64
                        drow = 64 * (1 - pr)
                        cps = ctx_ps[pr]
                        den = spool.tile([1, NS], F32, tag="den", name="den")
                        nc.vector.tensor_copy(den[:], cps[drow:drow + 1, :])
                        rec = spool.tile([1, NS], F32, tag="rec", name="rec")
                        nc.vector.reciprocal_approx_fast(rec[:], den[:])
                        bc = spool.tile([P, NS], F32, tag="bc", name="bc")
                        nc.gpsimd.partition_broadcast(bc[:], rec[:])
                        nc.vector.tensor_mul(
                            cT_j[j][pr0:pr0 + 64, ot, :],
                            cps[pr0:pr0 + 64, :],
                            bc[pr0:pr0 + 64, :],
                        )
                fns.append(norm_fn)
            pending.append(fns)

        def emit_block(ot, j):
            ctx_ps = {}
            for gi in range(len(K_GROUPS)):
                sc_group(ot, j, gi, ctx_ps)
                while len(pending) > 2:
                    flush_one()

        def emit_y(j):
            for qb in range(4):
                ysb = ypool.tile([P, D], F16, tag="y", name="ysb")
                yp = psS.tile([P, 3, NS], F32, tag="ps", name="yp")
                for ct in range(2):
                    for oh in range(2):
                        nc.tensor.matmul(
                            yp[:, oh],
                            cT_j[j][:, ct, qb * P:(qb + 1) * P],
                            wo_sb[:, ct, oh * NS:(oh + 1) * NS],
                            start=(ct == 0),
                            stop=(ct == 1),
                        )
                nc.vector.tensor_copy(
                    ysb[:].rearrange("p (a s) -> p a s", s=NS), yp[:, 0:2, :]
                )
                dq().dma_start(y.ap()[(j * 4 + qb) * P:(j * 4 + qb + 1) * P, :],
                               ysb[:])

        # ---- pipelined schedule ---------------------------------------
        dma_w(wk_sb, wkT)
        dma_w(wq_sb, wqT)
        xk0 = dma_x(xkT, 0)
        xq0 = dma_x(xqT, 0)

        proj_j(xk0, wk_sb, 0, kT_sb[:, :, 0:NS])
        proj_j(xk0, wk_sb, 1, kT_sb[:, :, NS:2 * NS])
        proj_j(xq0, wq_sb, 0, qT_j[0][:, :, :])

        xk1 = dma_x(xkT, 1)

        b00_ctx = {}
        sc_group(0, 0, 0, b00_ctx)      # needs kT cols 0:128
        sc_group(0, 0, 1, b00_ctx)      # cols 128:512
        proj_j(xk1, wk_sb, 0, kT_sb[:, :, 2 * NS:3 * NS])
        sc_group(0, 0, 2, b00_ctx)      # cols 512:896 (K j1 done above)
        proj_j(xk1, wk_sb, 1, kT_sb[:, :, 3 * NS:4 * NS])

        dma_w(wv_sb, wvT)
        xv[0] = dma_x(xvT, 0)

        sc_group(0, 0, 3, b00_ctx)      # cols 896:1280 (needs K j2)
        sc_group(0, 0, 4, b00_ctx)      # cols 1280:1664 (needs K j3)
        sc_group(0, 0, 5, b00_ctx)
        proj_j(xq0, wq_sb, 1, qT_j[1][:, :, :])

        dma_w(wo_sb, woT)
        xv[1] = dma_x(xvT, 1)

        vp(0)
        vp(1)

        xq1 = dma_x(xqT, 1)

        # block (1,0) woven with remaining V chunks; flush (0,0) PVs as
        # soon as the vt tiles they read are constructed (VP_NEED).
        b10_ctx = {}
        sc_group(1, 0, 0, b10_ctx)
        flush_one()                     # (0,0) G0: needs vp0
        vp(2)
        sc_group(1, 0, 1, b10_ctx)
        flush_one()                     # (0,0) G1: needs vp1
        vp(3)
        sc_group(1, 0, 2, b10_ctx)
        flush_one()                     # (0,0) G2: needs vp3
        vp(4)
        vp(5)
        sc_group(1, 0, 3, b10_ctx)
        flush_one()                     # (0,0) G3: needs vp4
        vp(6)
        sc_group(1, 0, 4, b10_ctx)
        flush_one()                     # (0,0) G4: needs vp6
        vp(7)
        sc_group(1, 0, 5, b10_ctx)
        flush_one()                     # (0,0) G5 + norm: needs vp7

        proj_j(xq1, wq_sb, 0, qT_j[2][:, :, :])
        proj_j(xq1, wq_sb, 1, qT_j[3][:, :, :])

        emit_block(0, 1)
        emit_block(1, 1)
        emit_y(0)
        emit_block(0, 2)
        emit_block(1, 2)
        emit_y(1)
        emit_block(0, 3)
        emit_block(1, 3)
        emit_y(2)
        while pending:
            flush_one()
        emit_y(3)

    nc.compile()
    return nc


_NC = None


def _get_nc():
    global _NC
    if _NC is None:
        _NC = build_nc()
    return _NC


def _interleave_w(w):
    # [NI*P, O] -> [P, NI*O] so the SBUF load DMA is contiguous
    n = w.shape[0] // P
    return np.ascontiguousarray(
        w.reshape(n, P, w.shape[1]).transpose(1, 0, 2).reshape(P, -1)
    ).astype(np.float16)


def _shard_inputs(Query, Key, Value, W_q, W_k, W_v, W_o):
    in_maps = []
    xT = {}
    for b in range(B):
        xT[b] = (
            np.ascontiguousarray(Query[b].T).astype(np.float16),
            np.ascontiguousarray(Key[b].T).astype(np.float16),
            np.ascontiguousarray(Value[b].T).astype(np.float16),
        )
    for b in range(B):
        for hg in range(4):
            r0 = hg * CL
            in_maps.append({
                "xqT": xT[b][0],
                "xkT": xT[b][1],
                "xvT": xT[b][2],
                "wqT": _interleave_w(np.ascontiguousarray(W_q[r0:r0 + CL, :].T)),
                "wkT": _interleave_w(np.ascontiguousarray(W_k[r0:r0 + CL, :].T)),
                "wvT": _interleave_w(np.ascontiguousarray(W_v[r0:r0 + CL, :].T)),
                "woT": _interleave_w(np.ascontiguousarray(W_o[:, r0:r0 + CL].T)),
            })
    return in_maps


def _reference_np(Query, Key, Value, mask, W_q, W_k, W_v, W_o):
    # Fallback for a non-trivial mask (never hit for the spec'd inputs).
    out = np.empty((B, S, D), dtype=np.float32)
    m = np.broadcast_to(mask, (1, 1, S, S))[0, 0]
    for b in range(B):
        Q = (Query[b] @ W_q.T).reshape(S, H, DK).transpose(1, 0, 2)
        K = (Key[b] @ W_k.T).reshape(S, H, DK).transpose(1, 0, 2)
        V = (Value[b] @ W_v.T).reshape(S, H, DK).transpose(1, 0, 2)
        ctx = np.empty((H, S, DK), dtype=np.float32)
        for h in range(H):
            s = (Q[h] @ K[h].T) / np.sqrt(DK)
            s = np.where(m == 0, -1e9, s)
            s -= s.max(axis=-1, keepdims=True)
            e = np.exp(s)
            ctx[h] = (e / e.sum(axis=-1, keepdims=True)) @ V[h]
        out[b] = ctx.transpose(1, 0, 2).reshape(S, D) @ W_o.T
    return out


def kernel(Query, Key, Value, mask, W_q, W_k, W_v, W_o, **_ignored):
    Query = np.asarray(Query, dtype=np.float32)
    Key = np.asarray(Key, dtype=np.float32)
    Value = np.asarray(Value, dtype=np.float32)
    W_q = np.asarray(W_q, dtype=np.float32)
    W_k = np.asarray(W_k, dtype=np.float32)
    W_v = np.asarray(W_v, dtype=np.float32)
    W_o = np.asarray(W_o, dtype=np.float32)

    if not np.all(np.asarray(mask) != 0):
        return _reference_np(Query, Key, Value, np.asarray(mask),
                             W_q, W_k, W_v, W_o)

    nc = _get_nc()
    in_maps = _shard_inputs(Query, Key, Value, W_q, W_k, W_v, W_o)
    res = bass_utils.run_bass_kernel_spmd(nc, in_maps, core_ids=list(range(8)))
    out = np.zeros((B, S, D), dtype=np.float32)
    for b in range(B):
        for hg in range(4):
            out[b] += res.results[b * 4 + hg]["y"].astype(np.float32)
    return out
